# revision 1
# baseline (speedup 1.0000x reference)
"""Trainium2 Bass kernel for the pairwise-MLP geometric convolution.

Reference computes, per batch z:
    rel[a,b]   = g[b] - g[a]
    h[a,b,:]   = relu(rel @ W1 + b1)                      [N,N,H]
    k[a,b,:]   = h @ W2 + b2  -> [N,N,C_OUT,C_IN]
    out[a,i]   = sum_{b,j} k[a,b,i,j] * f[b,j]

Key factorization (avoids materializing k, 537MB -> ~1MB):
    U = g @ W1  (so rel@W1 = U[b]-U[a])
    G[b,h,i]   = sum_j W2[h, i*C_IN+j] * f[b,j]
    out[a,i]   = sum_{b,h} relu(U[b,h]+b1[h]-U[a,h]) * G[b,h,i]
               + sum_j b2[i,j] * (sum_b f[b,j])

Sharding over 8 cores: z (2) x b-quarter (4). Each core computes the full
[i=32, a=256] transposed partial for its 64 b's; host sums quarters and
transposes. Contraction runs on the PE as 32 accumulating matmuls with
K=128 chunks of (b-pair x 64 h): stationary G chunk [128,32], moving
T chunk [128,256] built by one fused tensor_scalar (add bias, relu) per
chunk, spread across DVE/ACT/GPSIMD.

Hardware constraint honored throughout: a PE Matmult can carry at most ONE
sync-wait, so all small inputs arrive in a single packed DMA, and two
dummy matmuls make the PE observe the two g_sb gather DMAs before the
main accumulation chain.
"""

import os
import sys

import numpy as np

_TRN_REPO = "/opt/trn_rl_repo"
if _TRN_REPO not in sys.path:
    sys.path.insert(0, _TRN_REPO)

from contextlib import ExitStack

import concourse.bass as bass
import concourse.mybir as mybir
import concourse.tile as tile
from concourse.bass_utils import run_bass_kernel_spmd

from concourse.vector_clock import ScopedClock

# The walrus codegen used on the axon/PJRT path accepts at most ONE sync-wait
# per TPB instruction. Tile's kernel-tail drain aggregates a wait for every
# live semaphore onto a single Drain, which walrus rejects. Patch the tail to
# spread those waits across single-wait SP nops before an unadorned drain.
_orig_drain_and_barrier = tile.TileContext._drain_and_barrier


def _split_wait_drain_and_barrier(self, tick_clock, wait_clock):
    nc = self.nc
    probe = nc.sync.nop(nofuse=True)
    wait_clock.add_sem_waits(probe.ins, ScopedClock({None: tick_clock.global_clock}))
    si = probe.ins.sync_info
    waits = list(si.on_wait) if si is not None and si.on_wait else []
    if len(waits) > 1:
        probe.ins.sync_info = mybir.SyncInfo(on_wait=waits[:1], on_update=[])
        for w in waits[1:]:
            extra = nc.sync.nop(nofuse=True)
            extra.ins.sync_info = mybir.SyncInfo(on_wait=[w], on_update=[])
    nc.sync.drain()
    nc.all_engine_barrier()
    popped = nc._tile_sem_poison_stack.pop()
    assert popped is self._sem_poison
    nc.clear_and_free_semaphores(list(self.sems.allocated().values()))
    nc.all_engine_barrier()


tile.TileContext._drain_and_barrier = _split_wait_drain_and_barrier

F32 = mybir.dt.float32
# bf16 runs the PE at 1 cycle/row vs 4 for fp32; accumulation stays fp32 in
# PSUM. Only the big contraction operands (T, G) are bf16.
BF16 = mybir.dt.bfloat16
Z, N, C_IN, C_OUT, H = 2, 256, 32, 32, 64
BQ = 64          # b-points per core (N / 4 quarters)
NPAIR = BQ // 2  # 32 K-chunks of (2 b x 64 h) = 128

# packed small-input tensor layout (fp32): [64, PKW]
#   cols 0:256    fTfull   (parts 0:32)
#   cols 256:288  b2T      (parts 0:32)
#   col  288      b1c      (parts 0:64)
PKW = 289
# bf16 packed tensor (matmul operands), loaded as two DMAs (cols 0:MA,
# MA:MPW) so the U and first G' matmuls start before the whole tensor lands:
#   cols 0:64       fTq      (parts 0:32)
#   cols 64:320     gT       (parts 0:3)
#   cols 320:384    gTb      (parts 0:3)
#   cols 384:448    W1       (parts 0:3)
#   cols 448:2496   M2p
MPW = 2496
MA = 1472

# engine for each of the 32 T-chunk builds: v=vector(DVE), s=scalar(ACT),
# g=gpsimd. ACT carries the shared prep, DVE the G copies.
T_ENGINES = ["g", "s", "v", "g", "s", "g", "s", "v"] * 4
# PE warm-up matmuls between the G' matmuls and the main chain.
N_WARMERS = 16


def build_nc(debug: bool = False) -> bass.Bass:
    nc = bass.Bass("TRN2", target_bir_lowering=False, debug=debug, num_devices=8)

    m2p = nc.dram_tensor("M2p", [C_IN, MPW], BF16, kind="ExternalInput").ap()
    pk = nc.dram_tensor("pk", [H, PKW], F32, kind="ExternalInput").ap()
    outp = nc.dram_tensor("outp", [C_OUT, N], F32, kind="ExternalOutput").ap()

    with tile.TileContext(nc) as tc, ExitStack() as ctx:
        consts = ctx.enter_context(tc.tile_pool(name="consts", bufs=1))
        work = ctx.enter_context(tc.tile_pool(name="work", bufs=1))
        # bufs=NPAIR: every T tile gets its own slot, so no T-op ever waits
        # for a PE slot release (keeps every instruction at <=1 sync wait,
        # a walrus codegen hard limit).
        tpool = ctx.enter_context(tc.tile_pool(name="tpool", bufs=NPAIR))
        psum = ctx.enter_context(tc.tile_pool(name="psum", bufs=1, space="PSUM"))
        dpool = ctx.enter_context(tc.tile_pool(name="dpool", bufs=1, space="DRAM"))

        # ---- input loads. pk goes through the Pool SWDGE queue so the SP
        # HWDGE ring stays within 8 DMAs (no semaphore-lane reuse).
        m2p_sb = consts.tile([C_IN, MPW], BF16)
        nc.sync.dma_start(out=m2p_sb[:, 0:MA], in_=m2p[:, 0:MA])
        nc.sync.dma_start(out=m2p_sb[:, MA:MPW], in_=m2p[:, MA:MPW])
        pk_sb = consts.tile([H, PKW], F32)
        nc.gpsimd.dma_start(out=pk_sb, in_=pk)

        fTq_bf = m2p_sb[:, 0:64]
        gT_bf = m2p_sb[0:3, 64:320]
        gTb_bf = m2p_sb[0:3, 320:384]
        w1_bf = m2p_sb[0:3, 384:448]
        fTfull_sb = pk_sb[0:C_IN, 0:256]
        b2t_sb = pk_sb[0:C_IN, 256:288]
        b1_sb = pk_sb[0:H, 288:289]

        # First DVE / ACT ops must observe only the pk DMA semaphore.
        scol = work.tile([C_IN, 1], F32)
        nc.vector.tensor_reduce(out=scol, in_=fTfull_sb,
                                axis=mybir.AxisListType.X, op=mybir.AluOpType.add)
        s_bcast = work.tile([C_IN, N], BF16)
        nc.vector.tensor_scalar(out=s_bcast, in0=scol.broadcast_to([C_IN, N]),
                                scalar1=0.0, scalar2=None,
                                op0=mybir.AluOpType.add)
        b2t_bf = work.tile([C_IN, C_OUT], BF16)
        nc.vector.tensor_copy(b2t_bf, b2t_sb)

        # ---- U matmuls: U^T = W1^T @ g^T (bf16 in, fp32 accumulate).
        # Both U results share one PSUM bank, freeing a bank for the
        # warm-up matmuls.
        u_ps = psum.tile([H, N + BQ], F32)
        uaT_ps = u_ps[:, 0:N]
        ubT_ps = u_ps[:, N:N + BQ]
        nc.tensor.matmul(uaT_ps, lhsT=w1_bf, rhs=gT_bf, start=True, stop=True)
        nc.tensor.matmul(ubT_ps, lhsT=w1_bf, rhs=gTb_bf, start=True, stop=True)

        # All shared T-op inputs are produced on ACT so T consumers on any
        # engine need exactly one (ACT) wait. negUa duplicated on both
        # partition halves: [128, N].
        negua2 = work.tile([2 * H, N], F32)
        nc.scalar.activation(negua2[0:H, :], uaT_ps,
                             mybir.ActivationFunctionType.Copy, scale=-1.0)
        nc.scalar.activation(negua2[H:2 * H, :], uaT_ps,
                             mybir.ActivationFunctionType.Copy, scale=-1.0)

        # Ub + b1, then stacked by pair: ubT2[bl*H+h, p] = Ub[2p+bl, h] + b1[h]
        ubB = work.tile([H, BQ], F32)
        nc.vector.tensor_scalar(out=ubB, in0=ubT_ps, scalar1=b1_sb,
                                scalar2=None, op0=mybir.AluOpType.add)
        ubT2 = work.tile([2 * H, NPAIR], F32)
        ubB_r = ubB.rearrange("h (p two) -> h two p", two=2)
        nc.scalar.activation(ubT2[0:H, :], ubB_r[:, 0, :],
                             mybir.ActivationFunctionType.Copy)
        nc.scalar.activation(ubT2[H:2 * H, :], ubB_r[:, 1, :],
                             mybir.ActivationFunctionType.Copy)

        # ---- G: G'[b, h*32+i] = sum_j fTq[j,b] * M2p[j, h*32+i] ----
        g_ps = []
        for k in range(4):
            gp = psum.tile([BQ, 512], F32, name=f"g_ps{k}", tag=f"g_ps{k}")
            nc.tensor.matmul(gp, lhsT=fTq_bf,
                             rhs=m2p_sb[:, 448 + k * 512:448 + (k + 1) * 512],
                             start=True, stop=True)
            g_ps.append(gp)

        # PSUM -> SBUF on DVE (DMA cannot read PSUM), then bounce through
        # DRAM to regroup (b-pair, h) onto partitions.
        g_tmp = work.tile([BQ, H * C_OUT], BF16)
        for k in range(4):
            nc.vector.tensor_copy(g_tmp[:, k * 512:(k + 1) * 512], g_ps[k])
        g_sb = work.tile([2 * H, NPAIR, C_OUT], BF16)
        g_dram = dpool.tile([BQ, H * C_OUT], BF16)
        nc.sync.dma_start(out=g_dram, in_=g_tmp)
        # Two gathers split by p-half. Because 64 h * 32 i = 2048 = the
        # g_dram row stride, the (bl, h) pair merges into ONE uniform
        # stride-32 dim, keeping each side a legal 3D AP:
        #   src element (2p+bl, h*32+i) -> offset (bl*64+h)*32 + p*4096 + i
        g0 = g_dram[:, :]
        for ph in range(2):
            g_src = bass.AP(tensor=g0.tensor,
                            offset=g0.offset + ph * 16 * 4096,
                            ap=[[32, 2 * H], [4096, 16], [1, C_OUT]])
            nc.sync.dma_start(out=g_sb[:, 16 * ph:16 * (ph + 1), :],
                              in_=g_src)

        # ---- b2 bias term first in the acc group ----
        acc = psum.tile([C_OUT, N], F32)
        nc.tensor.matmul(acc, lhsT=b2t_bf, rhs=s_bcast, start=True, stop=False)

        scrap = psum.tile([C_OUT, 1], F32)

        def observe_gather(ph):
            # PE observes the p-half gather (one wait) so the following
            # main matmuls need only their T-tile wait.
            nc.tensor.matmul(scrap, lhsT=g_sb[:, 16 * ph, :],
                             rhs=g_sb[:, 16 * ph, 0:1],
                             start=True, stop=True)

        # ---- main contraction: acc[i, a] += G_p^T @ T_p ----
        # T-gated PE warm-up: warmer w consumes t_w as it is produced, so
        # the PE tracks T production (staying at high p-state) instead of
        # idling while the G gathers are in flight.
        warm_ps = psum.tile([C_OUT, N], F32)
        t_tiles = []
        for p in range(NPAIR):
            t_p = tpool.tile([2 * H, N], BF16, tag="T", name=f"t_{p}")
            t_tiles.append(t_p)
            eng = T_ENGINES[p]
            if eng == "s":
                nc.scalar.activation(t_p, negua2,
                                     mybir.ActivationFunctionType.Relu,
                                     bias=ubT2[:, p:p + 1], scale=1.0)
            else:
                e = nc.vector if eng == "v" else nc.gpsimd
                e.tensor_scalar(out=t_p, in0=negua2,
                                scalar1=ubT2[:, p:p + 1], scalar2=0.0,
                                op0=mybir.AluOpType.add,
                                op1=mybir.AluOpType.max)
            if p < N_WARMERS:
                nc.tensor.matmul(warm_ps, lhsT=t_p[0:C_IN, 0:C_OUT],
                                 rhs=t_p[0:C_IN, :], start=True, stop=True)
        for ph in range(2):
            observe_gather(ph)
            for p in range(16 * ph, 16 * (ph + 1)):
                nc.tensor.matmul(acc, lhsT=g_sb[:, p, :], rhs=t_tiles[p],
                                 start=False, stop=(p == NPAIR - 1))

        # ---- store ----
        out_sb = work.tile([C_OUT, N], F32)
        nc.scalar.activation(out_sb, acc, mybir.ActivationFunctionType.Copy)
        nc.sync.dma_start(out=outp, in_=out_sb)

    return nc


def shard_inputs(features, geometry, W1, b1, W2, b2) -> list[dict]:
    import ml_dtypes
    bf16 = ml_dtypes.bfloat16
    f = np.ascontiguousarray(np.asarray(features, np.float32))
    g = np.ascontiguousarray(np.asarray(geometry, np.float32))
    W1 = np.ascontiguousarray(np.asarray(W1, np.float32))
    b1 = np.ascontiguousarray(np.asarray(b1, np.float32))
    W2 = np.ascontiguousarray(np.asarray(W2, np.float32))
    b2 = np.ascontiguousarray(np.asarray(b2, np.float32))

    m2p = W2.reshape(H, C_OUT, C_IN).transpose(2, 0, 1).reshape(C_IN, H * C_OUT)
    b2t = np.ascontiguousarray(b2.reshape(C_OUT, C_IN).T)

    maps = []
    for core in range(8):
        z, q = divmod(core, 4)
        sl = slice(q * BQ, (q + 1) * BQ)
        pk = np.zeros((H, PKW), np.float32)
        pk[0:C_IN, 0:256] = f[z].T
        if q == 0:
            pk[0:C_IN, 256:288] = b2t
        pk[0:H, 288] = b1
        mp = np.zeros((C_IN, MPW), bf16)
        mp[:, 0:64] = f[z, sl].T.astype(bf16)
        mp[0:3, 64:320] = g[z].T.astype(bf16)
        mp[0:3, 320:384] = g[z, sl].T.astype(bf16)
        mp[0:3, 384:448] = W1.astype(bf16)
        mp[:, 448:2496] = m2p.astype(bf16)
        maps.append({"pk": pk, "M2p": mp})
    return maps


def unshard(parts: list[np.ndarray]) -> np.ndarray:
    out = np.empty((Z, N, C_OUT), np.float32)
    for z in range(Z):
        acc = parts[4 * z].astype(np.float32)
        for q in range(1, 4):
            acc = acc + parts[4 * z + q]
        out[z] = acc.T
    return out


def kernel(**inputs) -> np.ndarray:
    nc = build_nc(debug=False)
    in_maps = shard_inputs(**inputs)
    res = run_bass_kernel_spmd(nc, in_maps, list(range(8)))
    return unshard([r["outp"] for r in res.results])



# revision 20
# speedup vs baseline: 1.7963x; 1.7963x over previous
"""Trainium2 Bass kernel for the pairwise-MLP geometric convolution.

Reference computes, per batch z:
    rel[a,b]   = g[b] - g[a]
    h[a,b,:]   = relu(rel @ W1 + b1)                      [N,N,H]
    k[a,b,:]   = h @ W2 + b2  -> [N,N,C_OUT,C_IN]
    out[a,i]   = sum_{b,j} k[a,b,i,j] * f[b,j]

Factorization (k never materialized):
    U = g @ W1
    T[(b,h), a] = relu(U[b,h] + b1[h] - U[a,h])
    G[(b,h), i] = sum_j W2[h, i*C_IN+j] * f[b,j]
    out[a, i]   = sum_{(b,h)} T[(b,h),a] * G[(b,h),i]
                + sum_j b2[i,j] * (sum_b f[b,j])

Sharding over 8 cores: z (2) x b-quarter (4); each core computes the full
[a=256, i=32] partial for its 64 b's; host sums quarters.

Per-core dataflow (all matmul operands bf16, PSUM accumulation fp32):
  * K-chunks of 128 = (b-pair bl in {0,1}) x (h=64). 32 pairs.
  * G lands DIRECTLY in [(bl,h), (p,i)] PSUM layout via 64 small matmuls
    (lhsT = W2 packed [j, h]-per-i on host, rhs = even/odd-b features,
    out partition-offset 64*bl, column stride 32) -- no reshape DMAs.
  * Ub+b1 lands directly as [(bl,h), p] via one delta-expanded matmul
    (lhsT rows (bl',x) = [W1;b1] on the bl=bl' diagonal, rhs = paired g).
  * T tiles [128, 256] built by DVE (bf16 2x tensor_scalar, ~127ns),
    ACT (activation reading U from PSUM with scale=-1), and GPSIMD.
  * Main chain: out[a_half, i] += t_p[:, half].T @ g_p  -- 32-row matmuls,
    T stationary, so the PE streams 2048 rows instead of 8192.
  * b2 bias via rank-1 matmuls (ones x (b2^T @ sum_b f)).

Hardware constraint honored throughout: the walrus codegen accepts at most
ONE sync-wait per TPB instruction; consumers observe multi-engine deps
through single-wait dummy ops (Pool copies, PE scrap matmul).
"""

import os
import sys

import numpy as np

_TRN_REPO = "/opt/trn_rl_repo"
if _TRN_REPO not in sys.path:
    sys.path.insert(0, _TRN_REPO)

from contextlib import ExitStack

import concourse.bass as bass
import concourse.mybir as mybir
import concourse.tile as tile
from concourse.bass_utils import run_bass_kernel_spmd

from concourse.vector_clock import ScopedClock

# The walrus codegen used on the axon/PJRT path accepts at most ONE sync-wait
# per TPB instruction. Tile's kernel-tail drain aggregates a wait for every
# live semaphore onto a single Drain, which walrus rejects. Patch the tail to
# spread those waits across single-wait SP nops before an unadorned drain.
_orig_drain_and_barrier = tile.TileContext._drain_and_barrier


def _split_wait_drain_and_barrier(self, tick_clock, wait_clock):
    nc = self.nc
    probe = nc.sync.nop(nofuse=True)
    wait_clock.add_sem_waits(probe.ins, ScopedClock({None: tick_clock.global_clock}))
    si = probe.ins.sync_info
    waits = list(si.on_wait) if si is not None and si.on_wait else []
    if len(waits) > 1:
        probe.ins.sync_info = mybir.SyncInfo(on_wait=waits[:1], on_update=[])
        for w in waits[1:]:
            extra = nc.sync.nop(nofuse=True)
            extra.ins.sync_info = mybir.SyncInfo(on_wait=[w], on_update=[])
    nc.sync.drain()
    nc.all_engine_barrier()
    popped = nc._tile_sem_poison_stack.pop()
    assert popped is self._sem_poison
    nc.clear_and_free_semaphores(list(self.sems.allocated().values()))
    nc.all_engine_barrier()


tile.TileContext._drain_and_barrier = _split_wait_drain_and_barrier

F32 = mybir.dt.float32
BF16 = mybir.dt.bfloat16
Z, N, C_IN, C_OUT, H = 2, 256, 32, 32, 64
BQ = 64          # b-points per core (N / 4 quarters)
NPAIR = BQ // 2  # 32 K-chunks of (2 b x 64 h) = 128

# d1 [32, 832]: rows 0:3 -> gT (cols 0:256), W1dup (256:384);
#               rows 0:8 -> gb8 (384:416), w1bexp (416:544);
#               rows 0:32 -> f2e (544:576), f2o (576:608), fTb (608:672),
#               b2t (672:704), ones row 0 (704:832)
D1_P, D1_W = 32, 832
# wexp [32, 2048]: [j, i*64+h] = W2[h, i*32+j]
WEXP_P, WEXP_W = 32, 2048

# T-build schedule: engine of each build slot in production order.
# v=DVE (127ns), s=ACT (398ns), g=GPSIMD (451ns). ACT also does the
# ubT2 copy first and the G PSUM->SBUF copy mid-stream.
N_V, N_S, N_G = 23, 3, 6
N_WARM = 12       # p-state warmers (pre-U and into the G-matmul window)
N_TWARM = 0       # T-gated warmers between G matmuls and main chain


def _t_schedule():
    """Interleave T-builds across engines by estimated completion time.

    Returns a list of engine codes, one per pair, in estimated completion
    order (= PE consumption order).
    """
    slots = []
    for k in range(N_V):
        slots.append((4110 + 127 * (k + 1), "v"))
    for k in range(N_S):
        slots.append((5920 + 398 * (k + 1), "s"))
    for k in range(N_G):
        slots.append((4430 + 451 * (k + 1), "g"))
    slots.sort()
    return [e for _, e in slots]


def build_nc(debug: bool = False) -> bass.Bass:
    nc = bass.Bass("TRN2", target_bir_lowering=False, debug=debug, num_devices=8)

    d1 = nc.dram_tensor("d1", [D1_P, D1_W], BF16, kind="ExternalInput").ap()
    wexp = nc.dram_tensor("wexp", [WEXP_P, WEXP_W], BF16, kind="ExternalInput").ap()
    outp = nc.dram_tensor("outp", [128, 2 * C_OUT], BF16, kind="ExternalOutput").ap()

    sched = _t_schedule()

    with tile.TileContext(nc) as tc, ExitStack() as ctx:
        consts = ctx.enter_context(tc.tile_pool(name="consts", bufs=1))
        work = ctx.enter_context(tc.tile_pool(name="work", bufs=1))
        # every T tile gets its own slot so no T-op waits on a PE release
        tpool = ctx.enter_context(tc.tile_pool(name="tpool", bufs=NPAIR))
        psum = ctx.enter_context(tc.tile_pool(name="psum", bufs=1, space="PSUM"))

        # ---- input loads: both on the SP queue, d1 first
        d1_sb = consts.tile([D1_P, D1_W], BF16)
        nc.sync.dma_start(out=d1_sb, in_=d1)
        wexp_sb = consts.tile([WEXP_P, WEXP_W], BF16)
        nc.sync.dma_start(out=wexp_sb[:, 0:1024], in_=wexp[:, 0:1024])
        nc.sync.dma_start(out=wexp_sb[:, 1024:2048], in_=wexp[:, 1024:2048])

        gT = d1_sb[0:3, 0:256]
        w1dup = d1_sb[0:3, 256:384]
        gb8 = d1_sb[0:8, 384:416]
        w1bexp = d1_sb[0:8, 416:544]
        f2e = d1_sb[0:32, 544:576]
        f2o = d1_sb[0:32, 576:608]
        fTb = d1_sb[0:32, 608:672]
        b2t = d1_sb[0:32, 672:704]
        ones_row = d1_sb[0:1, 704:832]

        # ---- PE p-state warm-up: memset a row on Pool, then dummy matmuls
        wsrc = work.tile([1, 256], BF16)
        nc.gpsimd.memset(wsrc, 1.0)
        # PSUM tiles are access-chained by the dep tracker: give every
        # independently-consumed producer its own tile.
        wk_ps = psum.tile([128, 512], F32, name="wk_ps")
        u_ps = psum.tile([128, 256], F32, name="u_ps")
        ubbr_ps = psum.tile([128, 64], F32, name="ubbr_ps")
        br_ps_t = psum.tile([1, C_OUT], F32, name="br_ps_t")
        warm_ps = wk_ps[0:1, 0:256]
        for _ in range(N_WARM):
            nc.tensor.matmul(warm_ps, lhsT=wsrc[0:1, 0:1], rhs=wsrc,
                             start=True, stop=True)

        # ---- U matmuls (dup'd over both partition halves)
        uaT2_ps = u_ps
        nc.tensor.matmul(uaT2_ps, lhsT=w1dup, rhs=gT, start=True, stop=True)
        ub_ps = ubbr_ps[:, 0:NPAIR]
        nc.tensor.matmul(ub_ps, lhsT=w1bexp, rhs=gb8, start=True, stop=True)

        # ---- ubT2 (Ub + b1 in [(bl,h), p] layout) to SBUF on ACT
        ubT2 = work.tile([2 * H, NPAIR], F32)
        nc.scalar.activation(ubT2, ub_ps, mybir.ActivationFunctionType.Copy)

        # ---- small DVE chain: scol reduce, then negua2, then birow copy
        scol = work.tile([C_IN, 1], BF16)
        with nc.allow_low_precision(reason="bf16 matmul operand; one rounding"):
            nc.vector.tensor_reduce(out=scol, in_=fTb,
                                    axis=mybir.AxisListType.X,
                                    op=mybir.AluOpType.add)
        negua2 = work.tile([2 * H, N], BF16)
        nc.vector.tensor_scalar(out=negua2, in0=uaT2_ps, scalar1=-1.0,
                                scalar2=None, op0=mybir.AluOpType.mult)

        # b2 bias row: br = scol^T-contracted b2t  (biasrow mm waits DVE>=scol,
        # which also subsumes the d1b DMA for every later PE consumer of d1b)
        br_ps = br_ps_t
        nc.tensor.matmul(br_ps, lhsT=scol, rhs=b2t, start=True, stop=True)
        br_sb = work.tile([1, C_OUT], BF16)
        nc.scalar.activation(br_sb, br_ps, mybir.ActivationFunctionType.Copy)

        # ---- G: 64 matmuls land [(bl,h), (i,p)] directly in two PSUM banks
        # bank A holds i 0:16, bank B i 16:32; each matmul writes a
        # contiguous [64, 32] block at partition offset 64*bl.
        g_psA = psum.tile([2 * H, 512], F32, name="g_psA")
        g_psB = psum.tile([2 * H, 512], F32, name="g_psB")
        for i in range(C_OUT):
            lhs = wexp_sb[:, i * 64:(i + 1) * 64]
            bank = g_psA if i < 16 else g_psB
            il = i % 16
            for bl, f2 in ((0, f2e), (1, f2o)):
                gout = bank[bl * H:(bl + 1) * H, il * NPAIR:(il + 1) * NPAIR]
                nc.tensor.matmul(gout, lhsT=lhs, rhs=f2, start=True, stop=True)
        g_sb = work.tile([2 * H, NPAIR * C_OUT], BF16)
        nc.scalar.activation(g_sb[:, 0:512], g_psA,
                             mybir.ActivationFunctionType.Copy)
        nc.scalar.activation(g_sb[:, 512:1024], g_psB,
                             mybir.ActivationFunctionType.Copy)

        # ---- T builds. Observer 1-element copies make each engine see the
        # cross-engine inputs once, so T ops need at most one sync wait.
        t_tiles = [None] * NPAIR
        pool_dummy = work.tile([1, 2], F32)
        nc.gpsimd.tensor_copy(pool_dummy[0:1, 0:1], ubT2[0:1, 0:1])
        nc.gpsimd.tensor_copy(pool_dummy[0:1, 1:2], negua2[0:1, 0:1])
        dve_obs = work.tile([1, 1], F32)
        nc.vector.tensor_copy(dve_obs, ubT2[0:1, 0:1])
        act_obs = work.tile([1, 1], BF16)
        nc.scalar.activation(act_obs, negua2[0:1, 0:1],
                             mybir.ActivationFunctionType.Copy)

        for p, eng in enumerate(sched):
            t_p = tpool.tile([2 * H, N], BF16, tag="T", name=f"t_{p}")
            t_tiles[p] = t_p
            if eng == "s":
                nc.scalar.activation(t_p, negua2,
                                     mybir.ActivationFunctionType.Relu,
                                     bias=ubT2[:, p:p + 1], scale=1.0)
            else:
                e = nc.vector if eng == "v" else nc.gpsimd
                e.tensor_scalar(out=t_p, in0=negua2,
                                scalar1=ubT2[:, p:p + 1], scalar2=0.0,
                                op0=mybir.AluOpType.add,
                                op1=mybir.AluOpType.max)

        # ---- T-gated PE warmers (keep p-state up while g copy is in flight)
        tw_ps = wk_ps[0:C_IN, 256:320]
        for w in range(N_TWARM):
            t_w = t_tiles[w]
            nc.tensor.matmul(tw_ps, lhsT=t_w[0:C_IN, 0:C_IN],
                             rhs=t_w[0:C_IN, 0:64], start=True, stop=True)

        # ---- accumulator: rank-1 b2 bias first, then the main chain
        acc = wk_ps[:, 384:384 + 2 * C_OUT]
        for ah in range(2):
            nc.tensor.matmul(acc[:, ah * C_OUT:(ah + 1) * C_OUT],
                             lhsT=ones_row, rhs=br_sb,
                             start=True, stop=False, skip_group_check=True)

        # PE observes the g copy once; main matmuls then only wait their T
        scrap = wk_ps[0:1, 320:321]
        nc.tensor.matmul(scrap, lhsT=g_sb[:, 0:1], rhs=g_sb[:, 0:1],
                         start=True, stop=True)

        for p in range(NPAIR):
            t_p = t_tiles[p]
            g_p = g_sb[:, p::NPAIR]
            for ah in range(2):
                nc.tensor.matmul(acc[:, ah * C_OUT:(ah + 1) * C_OUT],
                                 lhsT=t_p[:, ah * 128:(ah + 1) * 128],
                                 rhs=g_p,
                                 start=False, stop=(p == NPAIR - 1),
                                 skip_group_check=True)

        # ---- store
        out_sb = work.tile([128, 2 * C_OUT], BF16)
        nc.vector.tensor_copy(out_sb, acc)
        nc.sync.dma_start(out=outp, in_=out_sb)

    return nc


def shard_inputs(features, geometry, W1, b1, W2, b2) -> list[dict]:
    import ml_dtypes
    bf16 = ml_dtypes.bfloat16
    f = np.ascontiguousarray(np.asarray(features, np.float32))
    g = np.ascontiguousarray(np.asarray(geometry, np.float32))
    W1 = np.ascontiguousarray(np.asarray(W1, np.float32))
    b1 = np.ascontiguousarray(np.asarray(b1, np.float32))
    W2 = np.ascontiguousarray(np.asarray(W2, np.float32))
    b2 = np.ascontiguousarray(np.asarray(b2, np.float32))

    # wexp[j, i*64+h] = W2[h, i*32+j]
    w2r = W2.reshape(H, C_OUT, C_IN)            # [h, i, j]
    wexp = np.ascontiguousarray(
        w2r.transpose(2, 1, 0).reshape(C_IN, C_OUT * H)).astype(bf16)

    # w1bexp[(bl',x), (bl,h)] = delta(bl,bl') * [W1;b1][x, h]
    w1b = np.concatenate([W1, b1[None, :]], axis=0)      # [4, H]
    w1bexp = np.zeros((8, 128), np.float32)
    w1bexp[0:4, 0:64] = w1b
    w1bexp[4:8, 64:128] = w1b

    w1dup = np.concatenate([W1, W1], axis=1)             # [3, 128]

    b2t = np.ascontiguousarray(b2.reshape(C_OUT, C_IN).T)  # [j, i]

    maps = []
    for core in range(8):
        z, q = divmod(core, 4)
        sl = slice(q * BQ, (q + 1) * BQ)
        fq = f[z, sl]                                    # [64, j]
        gq = g[z, sl]                                    # [64, 3]

        d1 = np.zeros((D1_P, D1_W), np.float32)
        d1[0:3, 0:256] = g[z].T
        d1[0:3, 256:384] = w1dup
        # gb8[(bl'*4+x), p] = g[2p+bl', x] for x<3, 1.0 for x=3
        gb8 = np.zeros((8, NPAIR), np.float32)
        gb8[0:3, :] = gq[0::2].T
        gb8[3, :] = 1.0
        gb8[4:7, :] = gq[1::2].T
        gb8[7, :] = 1.0
        d1[0:8, 384:416] = gb8
        d1[0:8, 416:544] = w1bexp
        d1[:, 544:576] = fq[0::2].T                      # f2e [j, p]
        d1[:, 576:608] = fq[1::2].T                      # f2o
        d1[:, 608:672] = fq.T                            # fTb
        d1[:, 672:704] = b2t
        d1[0, 704:832] = 1.0

        maps.append({
            "d1": d1.astype(bf16),
            "wexp": wexp,
        })
    return maps


def unshard(parts: list[np.ndarray]) -> np.ndarray:
    out = np.zeros((Z, N, C_OUT), np.float32)
    for z in range(Z):
        for q in range(4):
            p = np.asarray(parts[4 * z + q], np.float32)   # [128, 64]
            out[z, 0:128] += p[:, 0:C_OUT]
            out[z, 128:256] += p[:, C_OUT:2 * C_OUT]
    return out


def kernel(**inputs) -> np.ndarray:
    nc = build_nc(debug=False)
    in_maps = shard_inputs(**inputs)
    res = run_bass_kernel_spmd(nc, in_maps, list(range(8)))
    return unshard([r["outp"] for r in res.results])


# revision 25
# speedup vs baseline: 1.8865x; 1.0502x over previous
"""Trainium2 Bass kernel for the pairwise-MLP geometric convolution.

Reference computes, per batch z:
    rel[a,b]   = g[b] - g[a]
    h[a,b,:]   = relu(rel @ W1 + b1)                      [N,N,H]
    k[a,b,:]   = h @ W2 + b2  -> [N,N,C_OUT,C_IN]
    out[a,i]   = sum_{b,j} k[a,b,i,j] * f[b,j]

Factorization (k never materialized):
    U = g @ W1
    T[(b,h), a] = relu(U[b,h] + b1[h] - U[a,h])
    G[(b,h), i] = sum_j W2[h, i*C_IN+j] * f[b,j]
    out[a, i]   = sum_{(b,h)} T[(b,h),a] * G[(b,h),i]
                + sum_j b2[i,j] * (sum_b f[b,j])

Sharding over 8 cores: z (2) x b-quarter (4); each core computes the full
[a=256, i=32] partial for its 64 b's; host sums quarters.

Per-core dataflow (all matmul operands bf16, PSUM accumulation fp32):
  * K-chunks of 128 = (b-pair bl in {0,1}) x (h=64). 32 pairs.
  * G lands DIRECTLY in [(bl,h), (p,i)] PSUM layout via 64 small matmuls
    (lhsT = W2 packed [j, h]-per-i on host, rhs = even/odd-b features,
    out partition-offset 64*bl, column stride 32) -- no reshape DMAs.
  * Ub+b1 lands directly as [(bl,h), p] via one delta-expanded matmul
    (lhsT rows (bl',x) = [W1;b1] on the bl=bl' diagonal, rhs = paired g).
  * T tiles [128, 256] built by DVE (bf16 2x tensor_scalar, ~127ns),
    ACT (activation reading U from PSUM with scale=-1), and GPSIMD.
  * Main chain: out[a_half, i] += t_p[:, half].T @ g_p  -- 32-row matmuls,
    T stationary, so the PE streams 2048 rows instead of 8192.
  * b2 bias via rank-1 matmuls (ones x (b2^T @ sum_b f)).

Hardware constraint honored throughout: the walrus codegen accepts at most
ONE sync-wait per TPB instruction; consumers observe multi-engine deps
through single-wait dummy ops (Pool copies, PE scrap matmul).
"""

import os
import sys

import numpy as np

_TRN_REPO = "/opt/trn_rl_repo"
if _TRN_REPO not in sys.path:
    sys.path.insert(0, _TRN_REPO)

from contextlib import ExitStack

import concourse.bass as bass
import concourse.mybir as mybir
import concourse.tile as tile
from concourse.bass_utils import run_bass_kernel_spmd

from concourse.vector_clock import ScopedClock

# The walrus codegen used on the axon/PJRT path accepts at most ONE sync-wait
# per TPB instruction. Tile's kernel-tail drain aggregates a wait for every
# live semaphore onto a single Drain, which walrus rejects. Patch the tail to
# spread those waits across single-wait SP nops before an unadorned drain.
_orig_drain_and_barrier = tile.TileContext._drain_and_barrier


def _split_wait_drain_and_barrier(self, tick_clock, wait_clock):
    nc = self.nc
    probe = nc.sync.nop(nofuse=True)
    wait_clock.add_sem_waits(probe.ins, ScopedClock({None: tick_clock.global_clock}))
    si = probe.ins.sync_info
    waits = list(si.on_wait) if si is not None and si.on_wait else []
    if len(waits) > 1:
        probe.ins.sync_info = mybir.SyncInfo(on_wait=waits[:1], on_update=[])
        for w in waits[1:]:
            extra = nc.sync.nop(nofuse=True)
            extra.ins.sync_info = mybir.SyncInfo(on_wait=[w], on_update=[])
    nc.sync.drain()
    nc.all_engine_barrier()
    popped = nc._tile_sem_poison_stack.pop()
    assert popped is self._sem_poison
    nc.clear_and_free_semaphores(list(self.sems.allocated().values()))
    nc.all_engine_barrier()


tile.TileContext._drain_and_barrier = _split_wait_drain_and_barrier

# The Bass constructor registers four const APs via gpsimd.memset; Pool's Q7
# launch overhead puts ~0.4us of serial work ahead of the opening all-engine
# barrier. Reroute those preamble memsets to the (faster, otherwise idle) DVE
# queue. The barrier after them still guarantees completion.
_orig_bass_init = bass.Bass.__init__


def _patched_bass_init(self, *a, **k):
    self._in_preamble_init = True
    try:
        _orig_bass_init(self, *a, **k)
    finally:
        self._in_preamble_init = False


_orig_memset = bass.BassEitherVectorEngine.memset


def _patched_memset(self, ap, constant):
    b = getattr(self, "bass", None)
    if b is not None and getattr(b, "_in_preamble_init", False):
        return _orig_memset(b.vector, ap, constant)
    return _orig_memset(self, ap, constant)


bass.Bass.__init__ = _patched_bass_init
bass.BassEitherVectorEngine.memset = _patched_memset

F32 = mybir.dt.float32
BF16 = mybir.dt.bfloat16
Z, N, C_IN, C_OUT, H = 2, 256, 32, 32, 64
BQ = 64          # b-points per core (N / 4 quarters)
NPAIR = BQ // 2  # 32 K-chunks of (2 b x 64 h) = 128

# d1 [32, 832]: rows 0:3 -> gT (cols 0:256), W1dup (256:384);
#               rows 0:8 -> gb8 (384:416), w1bexp (416:544);
#               rows 0:32 -> f2e (544:576), f2o (576:608), fTb (608:672),
#               b2t (672:704), ones row 0 (704:832)
D1_P, D1_W = 32, 832
# wexp [32, 2048]: [j, i*64+h] = W2[h, i*32+j]
WEXP_P, WEXP_W = 32, 2048

# T-build schedule: engine of each build slot in production order.
# v=DVE (127ns), s=ACT (398ns), g=GPSIMD (451ns). ACT also does the
# ubT2 copy first and the G PSUM->SBUF copy mid-stream.
N_V, N_S, N_G = 23, 3, 6
N_WARM = 8        # p-state warmers
N_TWARM = 0       # T-gated warmers between G matmuls and main chain


def _t_schedule():
    """Interleave T-builds across engines by estimated completion time.

    Returns a list of engine codes, one per pair, in estimated completion
    order (= PE consumption order).
    """
    slots = []
    for k in range(N_V):
        slots.append((4260 + 127 * (k + 1), "v"))
    for k in range(N_S):
        slots.append((5670 + 398 * (k + 1), "s"))
    for k in range(N_G):
        slots.append((4500 + 451 * (k + 1), "g"))
    slots.sort()
    return [e for _, e in slots]


def build_nc(debug: bool = False) -> bass.Bass:
    nc = bass.Bass("TRN2", target_bir_lowering=False, debug=debug, num_devices=8)

    d1 = nc.dram_tensor("d1", [D1_P, D1_W], BF16, kind="ExternalInput").ap()
    wexp = nc.dram_tensor("wexp", [WEXP_P, WEXP_W], BF16, kind="ExternalInput").ap()
    outp = nc.dram_tensor("outp", [128, 2 * C_OUT], BF16, kind="ExternalOutput").ap()

    sched = _t_schedule()

    with tile.TileContext(nc) as tc, ExitStack() as ctx:
        consts = ctx.enter_context(tc.tile_pool(name="consts", bufs=1))
        work = ctx.enter_context(tc.tile_pool(name="work", bufs=1))
        # every T tile gets its own slot so no T-op waits on a PE release
        tpool = ctx.enter_context(tc.tile_pool(name="tpool", bufs=NPAIR))
        psum = ctx.enter_context(tc.tile_pool(name="psum", bufs=1, space="PSUM"))

        # ---- input loads: both on the SP queue, d1 first
        d1_sb = consts.tile([D1_P, D1_W], BF16)
        nc.sync.dma_start(out=d1_sb, in_=d1)
        wexp_sb = consts.tile([WEXP_P, WEXP_W], BF16)
        nc.sync.dma_start(out=wexp_sb[:, 0:1024], in_=wexp[:, 0:1024])
        nc.sync.dma_start(out=wexp_sb[:, 1024:2048], in_=wexp[:, 1024:2048])

        gT = d1_sb[0:3, 0:256]
        w1dup = d1_sb[0:3, 256:384]
        gb8 = d1_sb[0:8, 384:416]
        w1bexp = d1_sb[0:8, 416:544]
        f2e = d1_sb[0:32, 544:576]
        f2o = d1_sb[0:32, 576:608]
        fTb = d1_sb[0:32, 608:672]
        b2t = d1_sb[0:32, 672:704]
        ones_row = d1_sb[0:1, 704:832]

        # ---- PE p-state warm-up: memset a row on Pool, then dummy matmuls
        wsrc = work.tile([1, 256], BF16)
        nc.vector.memset(wsrc, 1.0)
        # PSUM tiles are access-chained by the dep tracker: give every
        # independently-consumed producer its own tile.
        wk_ps = psum.tile([128, 512], F32, name="wk_ps")
        u_ps = psum.tile([128, 256], F32, name="u_ps")
        ubbr_ps = psum.tile([128, 64], F32, name="ubbr_ps")
        br_ps_t = psum.tile([1, C_OUT], F32, name="br_ps_t")
        warm_ps = wk_ps[0:1, 0:256]
        for _ in range(N_WARM):
            nc.tensor.matmul(warm_ps, lhsT=wsrc[0:1, 0:1], rhs=wsrc,
                             start=True, stop=True)

        # ---- U matmuls (dup'd over both partition halves)
        uaT2_ps = u_ps
        nc.tensor.matmul(uaT2_ps, lhsT=w1dup, rhs=gT, start=True, stop=True)
        ub_ps = ubbr_ps[:, 0:NPAIR]
        nc.tensor.matmul(ub_ps, lhsT=w1bexp, rhs=gb8, start=True, stop=True)

        # ---- ubT2 (Ub + b1 in [(bl,h), p] layout) to SBUF on ACT
        ubT2 = work.tile([2 * H, NPAIR], F32)
        nc.scalar.activation(ubT2, ub_ps, mybir.ActivationFunctionType.Copy)

        # ---- small DVE chain: scol reduce, then negua2, then birow copy
        scol = work.tile([C_IN, 1], BF16)
        with nc.allow_low_precision(reason="bf16 matmul operand; one rounding"):
            nc.vector.tensor_reduce(out=scol, in_=fTb,
                                    axis=mybir.AxisListType.X,
                                    op=mybir.AluOpType.add)
        negua2 = work.tile([2 * H, N], BF16)
        nc.vector.tensor_scalar(out=negua2, in0=uaT2_ps, scalar1=-1.0,
                                scalar2=None, op0=mybir.AluOpType.mult)

        # b2 bias row: br = scol^T-contracted b2t  (biasrow mm waits DVE>=scol,
        # which also subsumes the d1b DMA for every later PE consumer of d1b)
        br_ps = br_ps_t
        nc.tensor.matmul(br_ps, lhsT=scol, rhs=b2t, start=True, stop=True)
        br_sb = work.tile([1, C_OUT], BF16)
        nc.scalar.activation(br_sb, br_ps, mybir.ActivationFunctionType.Copy)

        # ---- G: 64 matmuls land [(bl,h), (i,p)] directly in two PSUM banks
        # bank A holds i 0:16, bank B i 16:32; each matmul writes a
        # contiguous [64, 32] block at partition offset 64*bl.
        g_psA = psum.tile([2 * H, 512], F32, name="g_psA")
        g_psB = psum.tile([2 * H, 512], F32, name="g_psB")
        for i in range(C_OUT):
            lhs = wexp_sb[:, i * 64:(i + 1) * 64]
            bank = g_psA if i < 16 else g_psB
            il = i % 16
            for bl, f2 in ((0, f2e), (1, f2o)):
                gout = bank[bl * H:(bl + 1) * H, il * NPAIR:(il + 1) * NPAIR]
                nc.tensor.matmul(gout, lhsT=lhs, rhs=f2, start=True, stop=True)
        g_sb = work.tile([2 * H, NPAIR * C_OUT], BF16)
        nc.scalar.activation(g_sb[:, 0:512], g_psA,
                             mybir.ActivationFunctionType.Copy)
        nc.scalar.activation(g_sb[:, 512:1024], g_psB,
                             mybir.ActivationFunctionType.Copy)

        # ---- T builds. Observer 1-element copies make each engine see the
        # cross-engine inputs once, so T ops need at most one sync wait.
        t_tiles = [None] * NPAIR
        pool_dummy = work.tile([1, 2], F32)
        nc.gpsimd.tensor_copy(pool_dummy[0:1, 0:1], ubT2[0:1, 0:1])
        nc.gpsimd.tensor_copy(pool_dummy[0:1, 1:2], negua2[0:1, 0:1])
        dve_obs = work.tile([1, 1], F32)
        nc.vector.tensor_copy(dve_obs, ubT2[0:1, 0:1])
        act_obs = work.tile([1, 1], BF16)
        nc.scalar.activation(act_obs, negua2[0:1, 0:1],
                             mybir.ActivationFunctionType.Copy)

        for p, eng in enumerate(sched):
            t_p = tpool.tile([2 * H, N], BF16, tag="T", name=f"t_{p}")
            t_tiles[p] = t_p
            if eng == "s":
                nc.scalar.activation(t_p, negua2,
                                     mybir.ActivationFunctionType.Relu,
                                     bias=ubT2[:, p:p + 1], scale=1.0)
            else:
                e = nc.vector if eng == "v" else nc.gpsimd
                e.tensor_scalar(out=t_p, in0=negua2,
                                scalar1=ubT2[:, p:p + 1], scalar2=0.0,
                                op0=mybir.AluOpType.add,
                                op1=mybir.AluOpType.max)

        # ---- T-gated PE warmers (keep p-state up while g copy is in flight)
        tw_ps = wk_ps[0:C_IN, 256:320]
        for w in range(N_TWARM):
            t_w = t_tiles[w]
            nc.tensor.matmul(tw_ps, lhsT=t_w[0:C_IN, 0:C_IN],
                             rhs=t_w[0:C_IN, 0:64], start=True, stop=True)

        # ---- accumulator: rank-1 b2 bias first, then the main chain
        acc = wk_ps[:, 384:384 + 2 * C_OUT]
        for ah in range(2):
            nc.tensor.matmul(acc[:, ah * C_OUT:(ah + 1) * C_OUT],
                             lhsT=ones_row, rhs=br_sb,
                             start=True, stop=False, skip_group_check=True)

        # PE observes the g copy once; main matmuls then only wait their T
        scrap = wk_ps[0:1, 320:321]
        nc.tensor.matmul(scrap, lhsT=g_sb[:, 0:1], rhs=g_sb[:, 0:1],
                         start=True, stop=True)

        for p in range(NPAIR):
            t_p = t_tiles[p]
            g_p = g_sb[:, p::NPAIR]
            for ah in range(2):
                nc.tensor.matmul(acc[:, ah * C_OUT:(ah + 1) * C_OUT],
                                 lhsT=t_p[:, ah * 128:(ah + 1) * 128],
                                 rhs=g_p,
                                 start=False, stop=(p == NPAIR - 1),
                                 skip_group_check=True)

        # ---- store
        out_sb = work.tile([128, 2 * C_OUT], BF16)
        nc.vector.tensor_copy(out_sb, acc)
        nc.sync.dma_start(out=outp, in_=out_sb)

    return nc


def shard_inputs(features, geometry, W1, b1, W2, b2) -> list[dict]:
    import ml_dtypes
    bf16 = ml_dtypes.bfloat16
    f = np.ascontiguousarray(np.asarray(features, np.float32))
    g = np.ascontiguousarray(np.asarray(geometry, np.float32))
    W1 = np.ascontiguousarray(np.asarray(W1, np.float32))
    b1 = np.ascontiguousarray(np.asarray(b1, np.float32))
    W2 = np.ascontiguousarray(np.asarray(W2, np.float32))
    b2 = np.ascontiguousarray(np.asarray(b2, np.float32))

    # wexp[j, i*64+h] = W2[h, i*32+j]
    w2r = W2.reshape(H, C_OUT, C_IN)            # [h, i, j]
    wexp = np.ascontiguousarray(
        w2r.transpose(2, 1, 0).reshape(C_IN, C_OUT * H)).astype(bf16)

    # w1bexp[(bl',x), (bl,h)] = delta(bl,bl') * [W1;b1][x, h]
    w1b = np.concatenate([W1, b1[None, :]], axis=0)      # [4, H]
    w1bexp = np.zeros((8, 128), np.float32)
    w1bexp[0:4, 0:64] = w1b
    w1bexp[4:8, 64:128] = w1b

    w1dup = np.concatenate([W1, W1], axis=1)             # [3, 128]

    b2t = np.ascontiguousarray(b2.reshape(C_OUT, C_IN).T)  # [j, i]

    maps = []
    for core in range(8):
        z, q = divmod(core, 4)
        sl = slice(q * BQ, (q + 1) * BQ)
        fq = f[z, sl]                                    # [64, j]
        gq = g[z, sl]                                    # [64, 3]

        d1 = np.zeros((D1_P, D1_W), np.float32)
        d1[0:3, 0:256] = g[z].T
        d1[0:3, 256:384] = w1dup
        # gb8[(bl'*4+x), p] = g[2p+bl', x] for x<3, 1.0 for x=3
        gb8 = np.zeros((8, NPAIR), np.float32)
        gb8[0:3, :] = gq[0::2].T
        gb8[3, :] = 1.0
        gb8[4:7, :] = gq[1::2].T
        gb8[7, :] = 1.0
        d1[0:8, 384:416] = gb8
        d1[0:8, 416:544] = w1bexp
        d1[:, 544:576] = fq[0::2].T                      # f2e [j, p]
        d1[:, 576:608] = fq[1::2].T                      # f2o
        d1[:, 608:672] = fq.T                            # fTb
        d1[:, 672:704] = b2t
        d1[0, 704:832] = 1.0

        maps.append({
            "d1": d1.astype(bf16),
            "wexp": wexp,
        })
    return maps


def unshard(parts: list[np.ndarray]) -> np.ndarray:
    out = np.zeros((Z, N, C_OUT), np.float32)
    for z in range(Z):
        for q in range(4):
            p = np.asarray(parts[4 * z + q], np.float32)   # [128, 64]
            out[z, 0:128] += p[:, 0:C_OUT]
            out[z, 128:256] += p[:, C_OUT:2 * C_OUT]
    return out


def kernel(**inputs) -> np.ndarray:
    nc = build_nc(debug=False)
    in_maps = shard_inputs(**inputs)
    res = run_bass_kernel_spmd(nc, in_maps, list(range(8)))
    return unshard([r["outp"] for r in res.results])
